# revision 7
# baseline (speedup 1.0000x reference)
"""Multi-head attention + residual + LayerNorm + output projection on 8 TRN2
NeuronCores.

Reference computation (B=4, S=1024, D=1024, H=16, DK=64, fp32):
    q/k/v = X @ W + b ; attn = softmax(q k^T / 8) ; x = attn v
    x = LN(x + query) * gamma + beta ; out = x @ Wo + bo
    returns (out, attn)

Sharding: sequence-parallel, zero collectives. Core c owns batch b = c//2 and
query rows [512*(c%2), 512*(c%2)+512) of that batch (i = 512 rows/core). Each
core computes k/v for its full batch (1024 rows), all 16 heads, then attention,
LayerNorm and the output projection for its own rows.

On-chip layout is feature-major ("transposed") end to end, which makes every
matmul a natural lhsT/rhs pair with zero on-chip transposes:
    qT[f, i] = sum_d Wq[d, f] queryT[d, i]       (lhsT=Wq tile, rhs=queryT)
    ST[j, i] = sum_d kT[d, j] qT[d, i]  per head (K=64 slice of a feat tile)
    PT[j, i] = exp(ST / 8)                        (ScalarE, PSUM->SBUF)
    x/rowsum via one attnV matmul: lhsT=[v | 1]   (ones column => softmax sums)
    LN stats via ones-vector matmuls over xT      (partition-axis reduction)
    out[i, f] = sum_d xnT[d, i] Wo[d, f]          (lhsT=xnT tile, rhs=Wo)
attn leaves the chip as [h, j, i] and the host transposes to [h, i, j].

All matmuls run in float32r (full PE rate at free-dim >=256 vs 4x slower fp32;
~1e-3 relative rounding, well inside the accuracy budget).

Per-head epilogues (reciprocal -> broadcast matmul -> normalize -> DMA) are
software-pipelined one head behind the matmul stream so the in-order PE queue
never waits on VectorE/ScalarE; softmax reciprocals run on ScalarE as
exp(-ln(s)); attn normalization alternates between VectorE and GpSimd.
"""
import sys

if "/opt/trn_rl_repo" not in sys.path:
    sys.path.insert(0, "/opt/trn_rl_repo")

import numpy as np

B, S, D, H = 4, 1024, 1024, 16
DK = D // H          # 64
N_CORES = 8
I = 512              # query rows per core
EPS = 1e-6
P = 128
NT = D // P          # 8 feature/contraction tiles
NJ = S // P          # 8 key-row tiles

_cache = {}


def _build():
    import concourse.bacc as bacc
    import concourse.tile as tile
    from concourse import mybir

    F32 = mybir.dt.float32
    F32R = mybir.dt.float32r
    AF = mybir.ActivationFunctionType
    ADD = mybir.AluOpType.add
    SUB = mybir.AluOpType.subtract
    MULT = mybir.AluOpType.mult

    nc = bacc.Bacc("TRN2", target_bir_lowering=False, debug=False,
                   num_devices=N_CORES)

    # ---- DRAM I/O (inputs fp32 bits; f32r declared so PE rounds on read) ----
    qT_d = nc.dram_tensor("qT", [D, I], F32R, kind="ExternalInput")
    kT_d = nc.dram_tensor("kT", [D, S], F32R, kind="ExternalInput")
    vT_d = nc.dram_tensor("vT", [D, S], F32R, kind="ExternalInput")
    wq_d = nc.dram_tensor("wq", [D, D], F32R, kind="ExternalInput")
    wk_d = nc.dram_tensor("wk", [D, D], F32R, kind="ExternalInput")
    wv_d = nc.dram_tensor("wv", [D, D], F32R, kind="ExternalInput")
    wo_d = nc.dram_tensor("wo", [D, D], F32R, kind="ExternalInput")
    bq_d = nc.dram_tensor("bq", [D], F32, kind="ExternalInput")
    bk_d = nc.dram_tensor("bk", [D], F32, kind="ExternalInput")
    bv_d = nc.dram_tensor("bv", [D], F32, kind="ExternalInput")
    bo_d = nc.dram_tensor("bo", [D], F32, kind="ExternalInput")
    gamma_d = nc.dram_tensor("gamma", [D], F32, kind="ExternalInput")
    beta_d = nc.dram_tensor("beta", [D], F32, kind="ExternalInput")
    out_d = nc.dram_tensor("out", [I, D], F32, kind="ExternalOutput")
    attnT_d = nc.dram_tensor("attnT", [H, S, I], F32, kind="ExternalOutput")

    wq_r = wq_d.rearrange("(t p) f -> p t f", p=P)
    wk_r = wk_d.rearrange("(t p) f -> p t f", p=P)
    wv_r = wv_d.rearrange("(t p) f -> p t f", p=P)
    wo_r = wo_d.rearrange("(t p) f -> p t f", p=P)

    with tile.TileContext(nc) as tc, \
         tc.tile_pool(name="const", bufs=1) as cpool, \
         tc.tile_pool(name="res", bufs=1) as rpool, \
         tc.tile_pool(name="qin", bufs=1) as qin_pool:

        # ---- constants ----
        ones_t = cpool.tile([P, P], F32R)
        nc.vector.memset(ones_t[:].bitcast(F32), 1.0)
        bq_sb = cpool.tile([P, NT], F32)
        bk_sb = cpool.tile([P, NT], F32)
        gam_sb = cpool.tile([P, NT], F32)
        bet_sb = cpool.tile([P, NT], F32)
        nc.sync.dma_start(out=bq_sb[:], in_=bq_d.rearrange("(t p) -> p t", p=P))
        nc.sync.dma_start(out=bk_sb[:], in_=bk_d.rearrange("(t p) -> p t", p=P))
        nc.sync.dma_start(out=gam_sb[:], in_=gamma_d.rearrange("(t p) -> p t", p=P))
        nc.sync.dma_start(out=bet_sb[:], in_=beta_d.rearrange("(t p) -> p t", p=P))
        bv_bc = cpool.tile([P, D], F32)
        bo_bc = cpool.tile([P, D], F32)
        nc.sync.dma_start(out=bv_bc[:], in_=bv_d.ap()[None, :].to_broadcast((P, D)))
        nc.sync.dma_start(out=bo_bc[:], in_=bo_d.ap()[None, :].to_broadcast((P, D)))
        eps_t = cpool.tile([1, 1], F32)
        nc.vector.memset(eps_t[:], EPS)

        # ---- long-lived results ----
        qT_sb = rpool.tile([P, NT, I], F32R, name="qT_sb")
        kT_sb = rpool.tile([P, NT, S], F32R, name="kT_sb")
        # v with an interleaved ones column per head: lhsT [v_h | 1] makes the
        # attnV matmul also produce the softmax row sums (at out row 64).
        vaug = rpool.tile([P, NJ, H * (DK + 1)], F32R, name="vaug")
        nc.vector.memset(vaug[:].bitcast(F32), 1.0)

        qTin = qin_pool.tile([P, NT, I], F32R)
        nc.sync.dma_start(out=qTin[:], in_=qT_d.rearrange("(t p) i -> p t i", p=P))

        # ================= Phase 1: projections =================
        # W streamed in 2 MB half-chunks (amortizes DMA fixed cost, keeps the
        # PE fed); kT input fully resident, vT input in halves.
        with tc.tile_pool(name="kvin", bufs=1) as kvpool, \
             tc.tile_pool(name="w1", bufs=2) as wpool, \
             tc.tile_pool(name="ps1", bufs=4, space="PSUM") as ps1:

            kTin = kvpool.tile([P, NT, S], F32R, name="kTin")
            nc.sync.dma_start(out=kTin[:],
                              in_=kT_d.rearrange("(t p) j -> p t j", p=P))

            # q projection: qT_sb[f, i]
            for fh in range(2):
                wql = wpool.tile([P, NT, 512], F32R, name="w")
                nc.sync.dma_start(out=wql[:],
                                  in_=wq_r[:, :, fh * 512:(fh + 1) * 512])
                for fi in range(4):
                    ft = fh * 4 + fi
                    ps = ps1.tile([P, I], F32, name="mm")
                    for dt in range(NT):
                        nc.tensor.matmul(ps[:, :], wql[:, dt, fi * P:(fi + 1) * P],
                                         qTin[:, dt, :],
                                         start=(dt == 0), stop=(dt == NT - 1))
                    nc.vector.tensor_scalar(out=qT_sb[:, ft, :], in0=ps[:, :],
                                            scalar1=bq_sb[:, ft:ft + 1],
                                            scalar2=None, op0=ADD)

            # k projection: kT_sb[f, j]
            for fh in range(2):
                wkl = wpool.tile([P, NT, 512], F32R, name="w")
                nc.sync.dma_start(out=wkl[:],
                                  in_=wk_r[:, :, fh * 512:(fh + 1) * 512])
                for fi in range(4):
                    ft = fh * 4 + fi
                    for jc in range(2):
                        js = slice(jc * 512, (jc + 1) * 512)
                        ps = ps1.tile([P, 512], F32, name="mm")
                        for dt in range(NT):
                            nc.tensor.matmul(ps[:, :],
                                             wkl[:, dt, fi * P:(fi + 1) * P],
                                             kTin[:, dt, js],
                                             start=(dt == 0), stop=(dt == NT - 1))
                        nc.vector.tensor_scalar(out=kT_sb[:, ft, js], in0=ps[:, :],
                                                scalar1=bk_sb[:, ft:ft + 1],
                                                scalar2=None, op0=ADD)

            # v projection (natural [j, f]), scattered into vaug
            for jh in range(2):
                vTin = kvpool.tile([P, NT, 512], F32R, name="vTin")
                nc.sync.dma_start(
                    out=vTin[:],
                    in_=vT_d.rearrange("(t p) j -> p t j", p=P)[:, :,
                                                                jh * 512:(jh + 1) * 512])
                for fc in range(2):
                    wvl = wpool.tile([P, NT, 512], F32R, name="w")
                    nc.sync.dma_start(out=wvl[:],
                                      in_=wv_r[:, :, fc * 512:(fc + 1) * 512])
                    for jt in range(jh * 4, jh * 4 + 4):
                        ps = ps1.tile([P, 512], F32, name="mm")
                        lo = (jt - jh * 4) * P
                        for dt in range(NT):
                            nc.tensor.matmul(ps[:, :], vTin[:, dt, lo:lo + P],
                                             wvl[:, dt, :],
                                             start=(dt == 0), stop=(dt == NT - 1))
                        psv = ps[:, :].rearrange("p (h w) -> p h w", w=DK)
                        bvv = bv_bc[:, fc * 512:(fc + 1) * 512].rearrange(
                            "p (h w) -> p h w", w=DK)
                        h0 = 8 * fc
                        dst = vaug[:, jt, (DK + 1) * h0:(DK + 1) * (h0 + 8)]
                        dst = dst.rearrange("p (h w) -> p h w", w=DK + 1)
                        nc.vector.tensor_tensor(dst[:, :, 0:DK], psv[:, :, :],
                                                bvv[:, :, :], ADD)

        # ================= Phases 2-4 =================
        with tc.tile_pool(name="xres", bufs=1) as xpool, \
             tc.tile_pool(name="bcps", bufs=1, space="PSUM") as bc_ps, \
             tc.tile_pool(name="statps", bufs=1, space="PSUM") as stat_ps:

            xT_sb = xpool.tile([P, NT, I], F32R, name="xT_sb")
            xn_sb = xpool.tile([P, NT, I], F32R, name="xn_sb")

            # ---- Phase 2: attention, one-head-deep software pipeline ----
            s1 = stat_ps.tile([P, I], F32, name="s1")
            s2 = stat_ps.tile([P, I], F32, name="s2")

            with tc.tile_pool(name="pt", bufs=2) as pt_pool, \
                 tc.tile_pool(name="tmp2", bufs=2) as tpool, \
                 tc.tile_pool(name="stps", bufs=3, space="PSUM") as st_ps, \
                 tc.tile_pool(name="xps", bufs=2, space="PSUM") as x_ps_pool:

                def head_matmuls(h):
                    """STs + exp + attnV accumulation for head h."""
                    ft = h // 2
                    po = (h % 2) * 64
                    pt = pt_pool.tile([P, NJ, I], F32R, name="pt")
                    x_ps = x_ps_pool.tile([P, I], F32, name="xv")
                    for jt in range(NJ):
                        st = st_ps.tile([P, I], F32, name="st")
                        nc.tensor.matmul(
                            st[:, :],
                            kT_sb[po:po + DK, ft, jt * P:(jt + 1) * P],
                            qT_sb[po:po + DK, ft, :],
                            start=True, stop=True)
                        nc.scalar.activation(out=pt[:, jt, :], in_=st[:, :],
                                             func=AF.Exp, scale=0.125)
                        nc.tensor.matmul(
                            x_ps[0:DK + 1, :],
                            vaug[:, jt, (DK + 1) * h:(DK + 1) * (h + 1)],
                            pt[:, jt, :],
                            start=(jt == 0), stop=(jt == NJ - 1))
                    return pt, x_ps

                def head_epilogue(h, pt, x_ps):
                    """recip (ScalarE) -> broadcast (PE) -> normalize -> DMA,
                    plus interleaved residual + LN-stat work per dtile."""
                    ft = h // 2
                    even = (h % 2 == 0)
                    # 1/s = exp(-ln(s)) on ScalarE (keeps VectorE free)
                    rect = tpool.tile([P, I], F32R, name="rect")
                    nc.scalar.activation(out=rect[64:65, :], in_=x_ps[64:65, :],
                                         func=AF.Ln, scale=1.0)
                    nc.scalar.activation(out=rect[64:65, :], in_=rect[64:65, :],
                                         func=AF.Exp, scale=-1.0)
                    bc = bc_ps.tile([P, I], F32, name="bc")
                    nc.tensor.matmul(bc[:, :], ones_t[64:65, :],
                                     rect[64:65, :], start=True, stop=True)
                    bcs = tpool.tile([P, I], F32, name="bcs")
                    nc.vector.tensor_copy(bcs[:], bc[:, :])
                    # normalized head output into xT (partitions po:po+64);
                    # odd heads reach partitions 64:128 via a DMA hop (DVE
                    # cannot shift partitions).
                    if even:
                        nc.vector.tensor_tensor(xT_sb[0:DK, ft, :],
                                                x_ps[0:DK, :],
                                                bcs[0:DK, :], MULT)
                    else:
                        xodd = tpool.tile([DK, I], F32R, name="xodd")
                        nc.vector.tensor_tensor(xodd[:, :], x_ps[0:DK, :],
                                                bcs[0:DK, :], MULT)
                        nc.sync.dma_start(out=xT_sb[64:128, ft, :],
                                          in_=xodd[:, :])
                    # normalize PT in place; DVE and GpSimd alternate by head
                    eng = nc.vector if even else nc.gpsimd
                    eng.tensor_tensor(
                        pt[:, :, :], pt[:, :, :],
                        bcs[:, None, :].to_broadcast((P, NJ, I)), MULT)
                    nc.sync.dma_start(
                        out=attnT_d[h, :, :].rearrange("(t p) i -> p t i", p=P),
                        in_=pt[:, :, :].bitcast(F32))
                    # residual + LN stats for dtile ft once both heads are in
                    if not even:
                        nc.vector.tensor_tensor(xT_sb[:, ft, :], xT_sb[:, ft, :],
                                                qTin[:, ft, :], ADD)
                        xsq = tpool.tile([P, I], F32R, name="xsq")
                        with nc.allow_low_precision(reason="x^2 for LN stats"):
                            nc.vector.tensor_tensor(xsq[:, :], xT_sb[:, ft, :],
                                                    xT_sb[:, ft, :], MULT)
                        nc.tensor.matmul(s1[0:1, :], ones_t[:, 0:1],
                                         xT_sb[:, ft, :],
                                         start=(ft == 0), stop=(ft == NT - 1))
                        nc.tensor.matmul(s2[0:1, :], ones_t[:, 0:1], xsq[:, :],
                                         start=(ft == 0), stop=(ft == NT - 1))

                prev = None
                for h in range(H):
                    cur = head_matmuls(h)
                    if prev is not None:
                        head_epilogue(h - 1, *prev)
                    prev = cur
                head_epilogue(H - 1, *prev)

            # ---- Phase 3: LayerNorm scalars ----
            with tc.tile_pool(name="tmp3", bufs=1) as tpool:
                mu = tpool.tile([1, I], F32R, name="mu")
                with nc.allow_low_precision(reason="LN mean in f32r"):
                    nc.vector.tensor_scalar(out=mu[:], in0=s1[0:1, :],
                                            scalar1=1.0 / D, scalar2=None,
                                            op0=MULT)
                s2d = tpool.tile([1, I], F32, name="s2d")
                nc.vector.tensor_scalar(out=s2d[:], in0=s2[0:1, :],
                                        scalar1=1.0 / D, scalar2=None, op0=MULT)
                musq = tpool.tile([1, I], F32, name="musq")
                nc.vector.tensor_tensor(musq[:], mu[:].bitcast(F32),
                                        mu[:].bitcast(F32), MULT)
                var = tpool.tile([1, I], F32, name="var")
                nc.vector.tensor_tensor(var[:], s2d[:], musq[:], SUB)
                lnv = tpool.tile([1, I], F32, name="lnv")
                nc.scalar.activation(out=lnv[:], in_=var[:], func=AF.Ln,
                                     bias=eps_t[:], scale=1.0)
                rstd = tpool.tile([1, I], F32R, name="rstd")
                nc.scalar.activation(out=rstd[:], in_=lnv[:], func=AF.Exp,
                                     scale=-0.5)
                mu_bc = bc_ps.tile([P, I], F32, name="bc")
                nc.tensor.matmul(mu_bc[:, :], ones_t[0:1, :], mu[:],
                                 start=True, stop=True)
                mu_bcs = tpool.tile([P, I], F32, name="mubcs")
                nc.scalar.copy(out=mu_bcs[:], in_=mu_bc[:, :])
                rs_bc = bc_ps.tile([P, I], F32, name="bc")
                nc.tensor.matmul(rs_bc[:, :], ones_t[0:1, :], rstd[:],
                                 start=True, stop=True)
                rs_bcs = tpool.tile([P, I], F32, name="rsbcs")
                nc.scalar.copy(out=rs_bcs[:], in_=rs_bc[:, :])
                for dt in range(NT):
                    t1 = tpool.tile([P, I], F32, name="t1")
                    nc.vector.tensor_tensor(t1[:], xT_sb[:, dt, :], mu_bcs[:], SUB)
                    nc.vector.tensor_tensor(t1[:], t1[:], rs_bcs[:], MULT)
                    with nc.allow_low_precision(reason="LN out rounds to f32r"):
                        nc.vector.tensor_scalar(out=xn_sb[:, dt, :], in0=t1[:],
                                                scalar1=gam_sb[:, dt:dt + 1],
                                                scalar2=bet_sb[:, dt:dt + 1],
                                                op0=MULT, op1=ADD)

            # ---- Phase 4: output projection ----
            with tc.tile_pool(name="wo", bufs=2) as wo_pool, \
                 tc.tile_pool(name="tmp4", bufs=2) as tpool4, \
                 tc.tile_pool(name="ops", bufs=3, space="PSUM") as out_ps:
                for fc in range(2):
                    wol = wo_pool.tile([P, NT, 512], F32R, name="wo")
                    nc.sync.dma_start(out=wol[:],
                                      in_=wo_r[:, :, fc * 512:(fc + 1) * 512])
                    for ic in range(4):
                        ops = out_ps.tile([P, 512], F32, name="op")
                        for dt in range(NT):
                            nc.tensor.matmul(ops[:, :],
                                             xn_sb[:, dt, ic * P:(ic + 1) * P],
                                             wol[:, dt, :],
                                             start=(dt == 0), stop=(dt == NT - 1))
                        osb = tpool4.tile([P, 512], F32, name="osb")
                        nc.vector.tensor_tensor(osb[:], ops[:, :],
                                                bo_bc[:, fc * 512:(fc + 1) * 512],
                                                ADD)
                        nc.sync.dma_start(
                            out=out_d[ic * P:(ic + 1) * P, fc * 512:(fc + 1) * 512],
                            in_=osb[:])

    nc.finalize()
    return nc


def _get_nc():
    if "nc" not in _cache:
        _cache["nc"] = _build()
    return _cache["nc"]


def _in_maps(inputs):
    qf = np.ascontiguousarray(np.asarray(inputs["query"], np.float32)).reshape(B * S, D)
    kf = np.ascontiguousarray(np.asarray(inputs["key"], np.float32)).reshape(B * S, D)
    vf = np.ascontiguousarray(np.asarray(inputs["value"], np.float32)).reshape(B * S, D)
    shared = {
        "wq": np.ascontiguousarray(np.asarray(inputs["Wq"], np.float32)),
        "wk": np.ascontiguousarray(np.asarray(inputs["Wk"], np.float32)),
        "wv": np.ascontiguousarray(np.asarray(inputs["Wv"], np.float32)),
        "wo": np.ascontiguousarray(np.asarray(inputs["Wo"], np.float32)),
        "bq": np.asarray(inputs["bq"], np.float32),
        "bk": np.asarray(inputs["bk"], np.float32),
        "bv": np.asarray(inputs["bv"], np.float32),
        "bo": np.asarray(inputs["bo"], np.float32),
        "gamma": np.asarray(inputs["gamma"], np.float32),
        "beta": np.asarray(inputs["beta"], np.float32),
    }
    maps = []
    for b in range(B):
        rows = slice(b * S, (b + 1) * S)
        qTb = np.ascontiguousarray(qf[rows].T)
        kTb = np.ascontiguousarray(kf[rows].T)
        vTb = np.ascontiguousarray(vf[rows].T)
        for half in range(2):
            maps.append(dict(shared,
                             qT=np.ascontiguousarray(qTb[:, half * I:(half + 1) * I]),
                             kT=kTb, vT=vTb))
    return maps


def _run(inputs, trace=False, trace_kwargs=None):
    from concourse.bass_utils import run_bass_kernel_spmd
    nc = _get_nc()
    res = run_bass_kernel_spmd(nc, _in_maps(inputs),
                               core_ids=list(range(N_CORES)),
                               trace=trace, **(trace_kwargs or {}))
    out = np.empty((B * S, D), np.float32)
    attn = np.empty((B, H, S, S), np.float32)
    for c in range(N_CORES):
        b, half = c // 2, c % 2
        r0 = b * S + half * I
        out[r0:r0 + I] = res.results[c]["out"]
        attn[b, :, half * I:half * I + I, :] = \
            res.results[c]["attnT"].transpose(0, 2, 1)
    return (out.reshape(B, S, D), attn), res


def kernel(**inputs):
    (out, attn), _ = _run(inputs)
    return out, attn


# revision 9
# speedup vs baseline: 1.1026x; 1.1026x over previous
"""Multi-head attention + residual + LayerNorm + output projection on 8 TRN2
NeuronCores.

Reference computation (B=4, S=1024, D=1024, H=16, DK=64, fp32):
    q/k/v = X @ W + b ; attn = softmax(q k^T / 8) ; x = attn v
    x = LN(x + query) * gamma + beta ; out = x @ Wo + bo
    returns (out, attn)

Sharding: sequence-parallel, zero collectives. Core c owns batch b = c//2 and
query rows [512*(c%2), 512*(c%2)+512) of that batch (i = 512 rows/core). Each
core computes k/v for its full batch (1024 rows), all 16 heads, then attention,
LayerNorm and the output projection for its own rows.

On-chip layout is feature-major ("transposed") end to end, which makes every
matmul a natural lhsT/rhs pair with zero on-chip transposes:
    qT[f, i] = sum_d Wq[d, f] queryT[d, i]       (lhsT=Wq tile, rhs=queryT)
    ST[j, i] = sum_d kT[d, j] qT[d, i]  per head (K=64 slice of a feat tile)
    PT[j, i] = exp(ST / 8)                        (ScalarE, PSUM->SBUF)
    x/rowsum via one attnV matmul: lhsT=[v | 1]   (ones column => softmax sums)
    LN stats via ones-vector matmuls over xT      (partition-axis reduction)
    out[i, f] = sum_d xnT[d, i] Wo[d, f]          (lhsT=xnT tile, rhs=Wo)
attn leaves the chip as [h, j, i] and the host transposes to [h, i, j].

All matmuls run in float32r (full PE rate at free-dim >=256 vs 4x slower fp32;
~1e-3 relative rounding, well inside the accuracy budget).

Per-head epilogues (reciprocal -> broadcast matmul -> normalize -> DMA) are
software-pipelined one head behind the matmul stream so the in-order PE queue
never waits on VectorE/ScalarE; softmax reciprocals run on ScalarE as
exp(-ln(s)); attn normalization alternates between VectorE and GpSimd.
"""
import sys

if "/opt/trn_rl_repo" not in sys.path:
    sys.path.insert(0, "/opt/trn_rl_repo")

import numpy as np

B, S, D, H = 4, 1024, 1024, 16
DK = D // H          # 64
N_CORES = 8
I = 512              # query rows per core
EPS = 1e-6
P = 128
NT = D // P          # 8 feature/contraction tiles
NJ = S // P          # 8 key-row tiles

_cache = {}


def _build():
    import concourse.bacc as bacc
    import concourse.tile as tile
    from concourse import mybir

    F32 = mybir.dt.float32
    F32R = mybir.dt.float32r
    AF = mybir.ActivationFunctionType
    ADD = mybir.AluOpType.add
    SUB = mybir.AluOpType.subtract
    MULT = mybir.AluOpType.mult

    nc = bacc.Bacc("TRN2", target_bir_lowering=False, debug=False,
                   num_devices=N_CORES)

    # ---- DRAM I/O (inputs fp32 bits; f32r declared so PE rounds on read) ----
    qT_d = nc.dram_tensor("qT", [D, I], F32R, kind="ExternalInput")
    kT_d = nc.dram_tensor("kT", [D, S], F32R, kind="ExternalInput")
    vT_d = nc.dram_tensor("vT", [D, S], F32R, kind="ExternalInput")
    wq_d = nc.dram_tensor("wq", [D, D], F32R, kind="ExternalInput")
    wk_d = nc.dram_tensor("wk", [D, D], F32R, kind="ExternalInput")
    wv_d = nc.dram_tensor("wv", [D, D], F32R, kind="ExternalInput")
    wo_d = nc.dram_tensor("wo", [D, D], F32R, kind="ExternalInput")
    bq_d = nc.dram_tensor("bq", [D], F32, kind="ExternalInput")
    bk_d = nc.dram_tensor("bk", [D], F32, kind="ExternalInput")
    bv_d = nc.dram_tensor("bv", [D], F32, kind="ExternalInput")
    bo_d = nc.dram_tensor("bo", [D], F32, kind="ExternalInput")
    gamma_d = nc.dram_tensor("gamma", [D], F32, kind="ExternalInput")
    beta_d = nc.dram_tensor("beta", [D], F32, kind="ExternalInput")
    out_d = nc.dram_tensor("out", [I, D], F32, kind="ExternalOutput")
    attnT_d = nc.dram_tensor("attnT", [H, S, I], F32, kind="ExternalOutput")

    wq_r = wq_d.rearrange("(t p) f -> p t f", p=P)
    wk_r = wk_d.rearrange("(t p) f -> p t f", p=P)
    wv_r = wv_d.rearrange("(t p) f -> p t f", p=P)
    wo_r = wo_d.rearrange("(t p) f -> p t f", p=P)

    with tile.TileContext(nc) as tc, \
         tc.tile_pool(name="const", bufs=1) as cpool, \
         tc.tile_pool(name="res", bufs=1) as rpool, \
         tc.tile_pool(name="qin", bufs=1) as qin_pool:

        # ---- constants ----
        ones_t = cpool.tile([P, P], F32R)
        nc.vector.memset(ones_t[:].bitcast(F32), 1.0)
        bq_sb = cpool.tile([P, NT], F32)
        bk_sb = cpool.tile([P, NT], F32)
        gam_sb = cpool.tile([P, NT], F32)
        bet_sb = cpool.tile([P, NT], F32)
        nc.sync.dma_start(out=bq_sb[:], in_=bq_d.rearrange("(t p) -> p t", p=P))
        nc.sync.dma_start(out=bk_sb[:], in_=bk_d.rearrange("(t p) -> p t", p=P))
        nc.sync.dma_start(out=gam_sb[:], in_=gamma_d.rearrange("(t p) -> p t", p=P))
        nc.sync.dma_start(out=bet_sb[:], in_=beta_d.rearrange("(t p) -> p t", p=P))
        bv_bc = cpool.tile([P, D], F32)
        bo_bc = cpool.tile([P, D], F32)
        nc.sync.dma_start(out=bv_bc[:], in_=bv_d.ap()[None, :].to_broadcast((P, D)))
        nc.sync.dma_start(out=bo_bc[:], in_=bo_d.ap()[None, :].to_broadcast((P, D)))
        eps_t = cpool.tile([1, 1], F32)
        nc.vector.memset(eps_t[:], EPS)

        # ---- long-lived results ----
        qT_sb = rpool.tile([P, NT, I], F32R, name="qT_sb")
        kT_sb = rpool.tile([P, NT, S], F32R, name="kT_sb")
        # v with an interleaved ones column per head: lhsT [v_h | 1] makes the
        # attnV matmul also produce the softmax row sums (at out row 64).
        vaug = rpool.tile([P, NJ, H * (DK + 1)], F32R, name="vaug")
        nc.vector.memset(vaug[:].bitcast(F32), 1.0)

        qTin = qin_pool.tile([P, NT, I], F32R)
        nc.sync.dma_start(out=qTin[:], in_=qT_d.rearrange("(t p) i -> p t i", p=P))

        # ================= Phase 1: projections =================
        # W streamed in 2 MB half-chunks (amortizes DMA fixed cost, keeps the
        # PE fed); kT input fully resident, vT input in halves.
        with tc.tile_pool(name="kvin", bufs=1) as kvpool, \
             tc.tile_pool(name="w1", bufs=2) as wpool, \
             tc.tile_pool(name="ps1", bufs=4, space="PSUM") as ps1:

            kTin = kvpool.tile([P, NT, S], F32R, name="kTin")
            nc.sync.dma_start(out=kTin[:],
                              in_=kT_d.rearrange("(t p) j -> p t j", p=P))

            # q projection: qT_sb[f, i]
            for fh in range(2):
                wql = wpool.tile([P, NT, 512], F32R, name="w")
                nc.sync.dma_start(out=wql[:],
                                  in_=wq_r[:, :, fh * 512:(fh + 1) * 512])
                for fi in range(4):
                    ft = fh * 4 + fi
                    ps = ps1.tile([P, I], F32, name="mm")
                    for dt in range(NT):
                        nc.tensor.matmul(ps[:, :], wql[:, dt, fi * P:(fi + 1) * P],
                                         qTin[:, dt, :],
                                         start=(dt == 0), stop=(dt == NT - 1))
                    nc.vector.tensor_scalar(out=qT_sb[:, ft, :], in0=ps[:, :],
                                            scalar1=bq_sb[:, ft:ft + 1],
                                            scalar2=None, op0=ADD)

            # k projection: kT_sb[f, j]
            for fh in range(2):
                wkl = wpool.tile([P, NT, 512], F32R, name="w")
                nc.sync.dma_start(out=wkl[:],
                                  in_=wk_r[:, :, fh * 512:(fh + 1) * 512])
                for fi in range(4):
                    ft = fh * 4 + fi
                    for jc in range(2):
                        js = slice(jc * 512, (jc + 1) * 512)
                        ps = ps1.tile([P, 512], F32, name="mm")
                        for dt in range(NT):
                            nc.tensor.matmul(ps[:, :],
                                             wkl[:, dt, fi * P:(fi + 1) * P],
                                             kTin[:, dt, js],
                                             start=(dt == 0), stop=(dt == NT - 1))
                        nc.vector.tensor_scalar(out=kT_sb[:, ft, js], in0=ps[:, :],
                                                scalar1=bk_sb[:, ft:ft + 1],
                                                scalar2=None, op0=ADD)

            # v projection (natural [j, f]), scattered into vaug
            for jh in range(2):
                vTin = kvpool.tile([P, NT, 512], F32R, name="vTin")
                nc.sync.dma_start(
                    out=vTin[:],
                    in_=vT_d.rearrange("(t p) j -> p t j", p=P)[:, :,
                                                                jh * 512:(jh + 1) * 512])
                for fc in range(2):
                    wvl = wpool.tile([P, NT, 512], F32R, name="w")
                    nc.sync.dma_start(out=wvl[:],
                                      in_=wv_r[:, :, fc * 512:(fc + 1) * 512])
                    for jt in range(jh * 4, jh * 4 + 4):
                        ps = ps1.tile([P, 512], F32, name="mm")
                        lo = (jt - jh * 4) * P
                        for dt in range(NT):
                            nc.tensor.matmul(ps[:, :], vTin[:, dt, lo:lo + P],
                                             wvl[:, dt, :],
                                             start=(dt == 0), stop=(dt == NT - 1))
                        psv = ps[:, :].rearrange("p (h w) -> p h w", w=DK)
                        bvv = bv_bc[:, fc * 512:(fc + 1) * 512].rearrange(
                            "p (h w) -> p h w", w=DK)
                        h0 = 8 * fc
                        dst = vaug[:, jt, (DK + 1) * h0:(DK + 1) * (h0 + 8)]
                        dst = dst.rearrange("p (h w) -> p h w", w=DK + 1)
                        nc.vector.tensor_tensor(dst[:, :, 0:DK], psv[:, :, :],
                                                bvv[:, :, :], ADD)

        # ================= Phases 2-4 =================
        with tc.tile_pool(name="xres", bufs=1) as xpool, \
             tc.tile_pool(name="bcps", bufs=1, space="PSUM") as bc_ps, \
             tc.tile_pool(name="statps", bufs=1, space="PSUM") as stat_ps:

            xT_sb = xpool.tile([P, NT, I], F32R, name="xT_sb")

            # ---- Phase 2: attention, one-head-deep software pipeline ----
            s1 = stat_ps.tile([P, I], F32, name="s1")
            s2 = stat_ps.tile([P, I], F32, name="s2")

            with tc.tile_pool(name="pt", bufs=3) as pt_pool, \
                 tc.tile_pool(name="tmp2", bufs=2) as tpool, \
                 tc.tile_pool(name="stps", bufs=3, space="PSUM") as st_ps, \
                 tc.tile_pool(name="xps", bufs=2, space="PSUM") as x_ps_pool:

                def head_matmuls(h):
                    """STs + exp + attnV accumulation for head h."""
                    ft = h // 2
                    po = (h % 2) * 64
                    pt = pt_pool.tile([P, NJ, I], F32R, name="pt")
                    x_ps = x_ps_pool.tile([P, I], F32, name="xv")
                    def emit_st(jt):
                        st = st_ps.tile([P, I], F32, name="st")
                        nc.tensor.matmul(
                            st[:, :],
                            kT_sb[po:po + DK, ft, jt * P:(jt + 1) * P],
                            qT_sb[po:po + DK, ft, :],
                            start=True, stop=True)
                        nc.scalar.activation(out=pt[:, jt, :], in_=st[:, :],
                                             func=AF.Exp, scale=0.125)

                    def emit_xv(jt):
                        nc.tensor.matmul(
                            x_ps[0:DK + 1, :],
                            vaug[:, jt, (DK + 1) * h:(DK + 1) * (h + 1)],
                            pt[:, jt, :],
                            start=(jt == 0), stop=(jt == NJ - 1))

                    for jt in range(NJ):
                        emit_st(jt)
                        if jt >= 1:
                            emit_xv(jt - 1)
                    emit_xv(NJ - 1)
                    return pt, x_ps

                def head_epilogue(h, pt, x_ps):
                    """recip (ScalarE) -> broadcast (PE) -> normalize -> DMA,
                    plus interleaved residual + LN-stat work per dtile."""
                    ft = h // 2
                    even = (h % 2 == 0)
                    rect = tpool.tile([P, I], F32R, name="rect")
                    with nc.allow_low_precision(reason="softmax recip in f32r"):
                        nc.vector.reciprocal(rect[64:65, :], x_ps[64:65, :])
                    bc = bc_ps.tile([P, I], F32, name="bc")
                    nc.tensor.matmul(bc[:, :], ones_t[64:65, :],
                                     rect[64:65, :], start=True, stop=True)
                    bcs = tpool.tile([P, I], F32, name="bcs")
                    nc.scalar.copy(out=bcs[:], in_=bc[:, :])
                    # normalized head output into xT (partitions po:po+64);
                    # odd heads reach partitions 64:128 via a DMA hop (DVE
                    # cannot shift partitions).
                    if even:
                        nc.vector.tensor_tensor(xT_sb[0:DK, ft, :],
                                                x_ps[0:DK, :],
                                                bcs[0:DK, :], MULT)
                    else:
                        xodd = tpool.tile([DK, I], F32R, name="xodd")
                        nc.vector.tensor_tensor(xodd[:, :], x_ps[0:DK, :],
                                                bcs[0:DK, :], MULT)
                        nc.sync.dma_start(out=xT_sb[64:128, ft, :],
                                          in_=xodd[:, :])
                    # normalize PT in place, split across VectorE (reads the
                    # broadcast straight from PSUM) and GpSimd (SBUF copy);
                    # DMA each half out as soon as it is normalized.
                    hj = NJ // 2
                    nc.vector.tensor_tensor(
                        pt[:, 0:hj, :], pt[:, 0:hj, :],
                        bc[:, None, :].to_broadcast((P, hj, I)), MULT)
                    nc.sync.dma_start(
                        out=attnT_d[h, 0:hj * P, :].rearrange(
                            "(t p) i -> p t i", p=P),
                        in_=pt[:, 0:hj, :].bitcast(F32))
                    nc.gpsimd.tensor_tensor(
                        pt[:, hj:NJ, :], pt[:, hj:NJ, :],
                        bcs[:, None, :].to_broadcast((P, NJ - hj, I)), MULT)
                    nc.sync.dma_start(
                        out=attnT_d[h, hj * P:S, :].rearrange(
                            "(t p) i -> p t i", p=P),
                        in_=pt[:, hj:NJ, :].bitcast(F32))
                    # residual + LN stats for dtile ft once both heads are in
                    if not even:
                        nc.vector.tensor_tensor(xT_sb[:, ft, :], xT_sb[:, ft, :],
                                                qTin[:, ft, :], ADD)
                        xsq = tpool.tile([P, I], F32R, name="xsq")
                        with nc.allow_low_precision(reason="x^2 for LN stats"):
                            nc.vector.tensor_tensor(xsq[:, :], xT_sb[:, ft, :],
                                                    xT_sb[:, ft, :], MULT)
                        nc.tensor.matmul(s1[0:1, :], ones_t[:, 0:1],
                                         xT_sb[:, ft, :],
                                         start=(ft == 0), stop=(ft == NT - 1))
                        nc.tensor.matmul(s2[0:1, :], ones_t[:, 0:1], xsq[:, :],
                                         start=(ft == 0), stop=(ft == NT - 1))

                prev = None
                for h in range(H):
                    cur = head_matmuls(h)
                    if prev is not None:
                        head_epilogue(h - 1, *prev)
                    prev = cur
                head_epilogue(H - 1, *prev)

            # ---- Phase 3: LayerNorm scalars ----
            with tc.tile_pool(name="xn", bufs=1) as xn_pool:
              xn_sb = xn_pool.tile([P, NT, I], F32R, name="xn_sb")
              with tc.tile_pool(name="tmp3", bufs=1) as tpool:
                mu = tpool.tile([1, I], F32R, name="mu")
                with nc.allow_low_precision(reason="LN mean in f32r"):
                    nc.vector.tensor_scalar(out=mu[:], in0=s1[0:1, :],
                                            scalar1=1.0 / D, scalar2=None,
                                            op0=MULT)
                s2d = tpool.tile([1, I], F32, name="s2d")
                nc.vector.tensor_scalar(out=s2d[:], in0=s2[0:1, :],
                                        scalar1=1.0 / D, scalar2=None, op0=MULT)
                musq = tpool.tile([1, I], F32, name="musq")
                nc.vector.tensor_tensor(musq[:], mu[:].bitcast(F32),
                                        mu[:].bitcast(F32), MULT)
                var = tpool.tile([1, I], F32, name="var")
                nc.vector.tensor_tensor(var[:], s2d[:], musq[:], SUB)
                lnv = tpool.tile([1, I], F32, name="lnv")
                nc.scalar.activation(out=lnv[:], in_=var[:], func=AF.Ln,
                                     bias=eps_t[:], scale=1.0)
                rstd = tpool.tile([1, I], F32R, name="rstd")
                nc.scalar.activation(out=rstd[:], in_=lnv[:], func=AF.Exp,
                                     scale=-0.5)
                mu_bc = bc_ps.tile([P, I], F32, name="bc")
                nc.tensor.matmul(mu_bc[:, :], ones_t[0:1, :], mu[:],
                                 start=True, stop=True)
                mu_bcs = tpool.tile([P, I], F32, name="mubcs")
                nc.scalar.copy(out=mu_bcs[:], in_=mu_bc[:, :])
                rs_bc = bc_ps.tile([P, I], F32, name="bc")
                nc.tensor.matmul(rs_bc[:, :], ones_t[0:1, :], rstd[:],
                                 start=True, stop=True)
                rs_bcs = tpool.tile([P, I], F32, name="rsbcs")
                nc.scalar.copy(out=rs_bcs[:], in_=rs_bc[:, :])
                for dt in range(NT):
                    t1 = tpool.tile([P, I], F32, name="t1")
                    nc.vector.tensor_tensor(t1[:], xT_sb[:, dt, :], mu_bcs[:], SUB)
                    nc.vector.tensor_tensor(t1[:], t1[:], rs_bcs[:], MULT)
                    with nc.allow_low_precision(reason="LN out rounds to f32r"):
                        nc.vector.tensor_scalar(out=xn_sb[:, dt, :], in0=t1[:],
                                                scalar1=gam_sb[:, dt:dt + 1],
                                                scalar2=bet_sb[:, dt:dt + 1],
                                                op0=MULT, op1=ADD)

              # ---- Phase 4: output projection ----
              with tc.tile_pool(name="wo", bufs=2) as wo_pool, \
                   tc.tile_pool(name="tmp4", bufs=2) as tpool4, \
                   tc.tile_pool(name="ops", bufs=3, space="PSUM") as out_ps:
                for fc in range(2):
                    wol = wo_pool.tile([P, NT, 512], F32R, name="wo")
                    nc.sync.dma_start(out=wol[:],
                                      in_=wo_r[:, :, fc * 512:(fc + 1) * 512])
                    for ic in range(4):
                        ops = out_ps.tile([P, 512], F32, name="op")
                        for dt in range(NT):
                            nc.tensor.matmul(ops[:, :],
                                             xn_sb[:, dt, ic * P:(ic + 1) * P],
                                             wol[:, dt, :],
                                             start=(dt == 0), stop=(dt == NT - 1))
                        osb = tpool4.tile([P, 512], F32, name="osb")
                        nc.vector.tensor_tensor(osb[:], ops[:, :],
                                                bo_bc[:, fc * 512:(fc + 1) * 512],
                                                ADD)
                        nc.sync.dma_start(
                            out=out_d[ic * P:(ic + 1) * P, fc * 512:(fc + 1) * 512],
                            in_=osb[:])

    nc.finalize()
    return nc


def _get_nc():
    if "nc" not in _cache:
        _cache["nc"] = _build()
    return _cache["nc"]


def _in_maps(inputs):
    qf = np.ascontiguousarray(np.asarray(inputs["query"], np.float32)).reshape(B * S, D)
    kf = np.ascontiguousarray(np.asarray(inputs["key"], np.float32)).reshape(B * S, D)
    vf = np.ascontiguousarray(np.asarray(inputs["value"], np.float32)).reshape(B * S, D)
    shared = {
        "wq": np.ascontiguousarray(np.asarray(inputs["Wq"], np.float32)),
        "wk": np.ascontiguousarray(np.asarray(inputs["Wk"], np.float32)),
        "wv": np.ascontiguousarray(np.asarray(inputs["Wv"], np.float32)),
        "wo": np.ascontiguousarray(np.asarray(inputs["Wo"], np.float32)),
        "bq": np.asarray(inputs["bq"], np.float32),
        "bk": np.asarray(inputs["bk"], np.float32),
        "bv": np.asarray(inputs["bv"], np.float32),
        "bo": np.asarray(inputs["bo"], np.float32),
        "gamma": np.asarray(inputs["gamma"], np.float32),
        "beta": np.asarray(inputs["beta"], np.float32),
    }
    maps = []
    for b in range(B):
        rows = slice(b * S, (b + 1) * S)
        qTb = np.ascontiguousarray(qf[rows].T)
        kTb = np.ascontiguousarray(kf[rows].T)
        vTb = np.ascontiguousarray(vf[rows].T)
        for half in range(2):
            maps.append(dict(shared,
                             qT=np.ascontiguousarray(qTb[:, half * I:(half + 1) * I]),
                             kT=kTb, vT=vTb))
    return maps


def _run(inputs, trace=False, trace_kwargs=None):
    from concourse.bass_utils import run_bass_kernel_spmd
    nc = _get_nc()
    res = run_bass_kernel_spmd(nc, _in_maps(inputs),
                               core_ids=list(range(N_CORES)),
                               trace=trace, **(trace_kwargs or {}))
    out = np.empty((B * S, D), np.float32)
    attn = np.empty((B, H, S, S), np.float32)
    for c in range(N_CORES):
        b, half = c // 2, c % 2
        r0 = b * S + half * I
        out[r0:r0 + I] = res.results[c]["out"]
        attn[b, :, half * I:half * I + I, :] = \
            res.results[c]["attnT"].transpose(0, 2, 1)
    return (out.reshape(B, S, D), attn), res


def kernel(**inputs):
    (out, attn), _ = _run(inputs)
    return out, attn
